# revision 12
# baseline (speedup 1.0000x reference)
"""MiniGNN (DGCNN-style edge-conv) Trainium2 Bass kernel.

Math reduction used (BN is eval-mode with mean=0/var=1, LeakyReLU and the
BN affine (positive scale) are monotone, so max_k commutes with them and the
1x1 conv can be applied BEFORE the gather):

  per edge-conv layer (C -> C'):
    t   = A @ h            (A = s*W1, s = gamma/sqrt(1+eps))        [C', N]
    m   = max_k t[:, knn[n, k]]                                     [C', N]
    u   = B @ xc + bias    (B = s*(W2 - W1), bias = s*b)            [C', N]
    h'  = LeakyReLU(m + u)
  where xc[c, n] = h[n % C, c*(N/C) + n//C]  (the torch-faithful buffer
  reinterpret of h^T as [C, N]).

Per-core: batch b = core//2 (pairs duplicate work; each pair member computes
the full batch item).  Gather runs on GPSIMD ap_gather with a d=1
channel-major source replicated 128//C' times across partition blocks, each
block gathering a different quarter/eighth of the target points.
"""
import numpy as np

N = 16384
K = 20
B = 4
EPS = 1e-5
SLOPE = 0.2
NB = N // 128  # 128 chunks of 128 points


def _build_nc():
    import concourse.bass as bass
    import concourse.bacc as bacc
    import concourse.mybir as mybir
    from concourse.tile import TileContext
    from concourse import library_config

    f32 = mybir.dt.float32
    nc = bacc.Bacc("TRN2")

    # ---- DRAM I/O ----
    xT_d = nc.dram_tensor("xT", [128, NB * 3], f32, kind="ExternalInput")
    idx8_d = nc.dram_tensor("idx8", [128, 16 * 160], mybir.dt.int16, kind="ExternalInput")
    idx4_d = nc.dram_tensor("idx4", [128, 32 * 160], mybir.dt.int16, kind="ExternalInput")
    ident_d = nc.dram_tensor("ident", [128, 128], f32, kind="ExternalInput")
    # layer weights (transposed for matmul rhs) + broadcast biases
    LAY = [("p1", 3, 16), ("p2", 16, 16),
           ("e1", 16, 16), ("e2", 16, 16), ("e3", 16, 32), ("e4", 32, 32),
           ("e5", 32, 32), ("fin", 32, 32)]
    wd = {}
    for nm, c, cp in LAY:
        wd["wA" + nm] = nc.dram_tensor("wA" + nm, [c, cp], f32, kind="ExternalInput")
        wd["bb" + nm] = nc.dram_tensor("bb" + nm, [128, cp], f32, kind="ExternalInput")
        if nm.startswith("e"):
            wd["wB" + nm] = nc.dram_tensor("wB" + nm, [c, cp], f32, kind="ExternalInput")
    out_d = nc.dram_tensor("out", [N, 32], f32, kind="ExternalOutput")

    with TileContext(nc) as tc:
        with tc.tile_pool(name="gsrc", bufs=1) as gsrcp, \
             tc.tile_pool(name="big", bufs=1) as bigp, \
             tc.tile_pool(name="wpool", bufs=1) as wpool, \
             tc.tile_pool(name="dram", bufs=1, space="DRAM") as dramp, \
             tc.tile_pool(name="work", bufs=3) as work, \
             tc.tile_pool(name="gout", bufs=2) as goutp, \
             tc.tile_pool(name="psum", bufs=1, space="PSUM") as psum:

            Gsrc = gsrcp.tile([128, N], f32)           # gather source (replicated t)
            hT = bigp.tile([128, NB * 32], f32)        # h in [n-part, c] layout
            u_sb = bigp.tile([128, NB * 32], f32)      # u term per layer
            idx8 = bigp.tile([128, 16 * 160], mybir.dt.int16)
            idx4 = bigp.tile([128, 32 * 160], mybir.dt.int16)
            xT = bigp.tile([128, NB * 3], f32)
            xT_st = bigp.tile([128, NB * 3], f32)
            ident = wpool.tile([128, 128], f32)
            ident_st = wpool.tile([128, 128], f32)
            nc.sync.dma_start(out=idx8[:, :], in_=idx8_d[:, :])
            nc.sync.dma_start(out=idx4[:, :], in_=idx4_d[:, :])
            nc.sync.dma_start(out=xT_st[:, :], in_=xT_d[:, :])
            nc.sync.dma_start(out=ident_st[:, :], in_=ident_d[:, :])
            # PE-consumed tensors are staged through a DVE copy so every PE
            # instruction carries at most one (DVE) semaphore wait.
            nc.vector.tensor_copy(out=xT[:, :], in_=xT_st[:, :])
            nc.vector.tensor_copy(out=ident[:, :], in_=ident_st[:, :])
            W = {}
            for nm, c, cp in LAY:
                for wn, rows in (("wA", c), ("bb", 128)) + ((("wB", c),) if nm.startswith("e") else ()):
                    key = wn + nm
                    st = wpool.tile([rows, cp], f32, tag=key + "s", name=key + "s")
                    nc.sync.dma_start(out=st[:, :], in_=wd[key][:, :])
                    W[key] = wpool.tile([rows, cp], f32, tag=key, name=key)
                    nc.vector.tensor_copy(out=W[key][:, :], in_=st[:, :])

            hTd = [dramp.tile([N, 32], f32, tag="hTd0", name="hTd0"),
                   dramp.tile([N, 32], f32, tag="hTd1", name="hTd1")]

            def lrelu(out_ap, in_ap):
                nc.vector.scalar_tensor_tensor(
                    out=out_ap, in0=in_ap, scalar=SLOPE, in1=in_ap,
                    op0=mybir.AluOpType.mult, op1=mybir.AluOpType.max)

            # ---------- L0: pe1 + pe2 (pointwise convs) ----------
            for i in range(NB):
                p1 = psum.tile([3, 128], f32, tag="tr")
                nc.tensor.transpose(out=p1[:, :], in_=xT[:, i * 3:(i + 1) * 3],
                                    identity=ident[:, :])
                x3 = work.tile([3, 128], f32, tag="hc")
                nc.vector.tensor_copy(out=x3[:, :], in_=p1[:, :])
                pm = psum.tile([128, 16], f32, tag="mm")
                nc.tensor.matmul(out=pm[:, :], lhsT=x3[:, :], rhs=W["wAp1"][:, :],
                                 start=True, stop=True)
                t1 = work.tile([128, 16], f32, tag="t1")
                nc.vector.tensor_add(out=t1[:, :], in0=pm[:, :], in1=W["bbp1"][:, :16])
                lrelu(t1[:, :], t1[:, :])
                p2 = psum.tile([16, 128], f32, tag="tr")
                nc.tensor.transpose(out=p2[:, :], in_=t1[:, :], identity=ident[:, :])
                h1c = work.tile([16, 128], f32, tag="hc")
                nc.vector.tensor_copy(out=h1c[:, :], in_=p2[:, :])
                pm2 = psum.tile([128, 16], f32, tag="mm")
                nc.tensor.matmul(out=pm2[:, :], lhsT=h1c[:, :], rhs=W["wAp2"][:, :],
                                 start=True, stop=True)
                hw = hT[:, i * 16:(i + 1) * 16]
                nc.vector.tensor_add(out=hw, in0=pm2[:, :], in1=W["bbp2"][:, :16])
                lrelu(hw, hw)
                nc.sync.dma_start(out=hTd[0][i * 128:(i + 1) * 128, :16], in_=hw)

            # ---------- edge-conv layers ----------
            ECS = [("e1", 16, 16), ("e2", 16, 16), ("e3", 16, 32),
                   ("e4", 32, 32), ("e5", 32, 32)]
            for li, (nm, C, Cp) in enumerate(ECS):
                src_hTd = hTd[li % 2]
                dst_hTd = hTd[(li + 1) % 2]
                REP = 128 // Cp
                nch = N // REP // 128       # gather chunks per block
                idxt = idx8 if Cp == 16 else idx4

                # --- alpha: build t = A@h, channel-major, replicated ---
                for i in range(NB):
                    ptr = psum.tile([C, 128], f32, tag="tr")
                    nc.tensor.transpose(out=ptr[:, :], in_=hT[:, i * C:(i + 1) * C],
                                        identity=ident[:, :])
                    hc = work.tile([C, 128], f32, tag="hc")
                    nc.vector.tensor_copy(out=hc[:, :], in_=ptr[:, :])
                    pt = psum.tile([128, Cp], f32, tag="mm")
                    nc.tensor.matmul(out=pt[:, :], lhsT=hc[:, :], rhs=W["wA" + nm][:, :],
                                     start=True, stop=True)
                    tsb = work.tile([128, Cp], f32, tag="t1")
                    nc.vector.tensor_copy(out=tsb[:, :], in_=pt[:, :])
                    ptr2 = psum.tile([Cp, 128], f32, tag="tr2")
                    nc.tensor.transpose(out=ptr2[:, :], in_=tsb[:, :], identity=ident[:, :])
                    nc.vector.tensor_copy(out=Gsrc[0:Cp, i * 128:(i + 1) * 128], in_=ptr2[:, :])
                for rep in range(1, REP):
                    nc.sync.dma_start(out=Gsrc[rep * Cp:(rep + 1) * Cp, :], in_=Gsrc[0:Cp, :])

                # --- u term via M_g trick: u[gC+r, o] = sum_c h[r, c*(N/C)+g]*B[o,c]
                NG = N // C                  # number of g values
                PER = 128 // C               # g's per psum tile
                hTd_v = src_hTd[:, :C].rearrange("(c g) r -> c g r", c=C)
                for gb in range(NG // 128):
                    TMP = work.tile([C, 128 * C], f32, tag="tmp")
                    nc.sync.dma_start(
                        out=TMP[:, :].rearrange("q (g r) -> q g r", r=C),
                        in_=hTd_v[:, gb * 128:(gb + 1) * 128, :])
                    gpm = 32 // C  # g's per matmul (PE out base must be mult of 32)
                    for t in range(128 // PER):
                        pu = psum.tile([128, Cp], f32, tag="mmu")
                        for e in range(4):
                            gg = t * PER + e * gpm
                            nc.tensor.matmul(
                                out=pu[e * 32:(e + 1) * 32, :],
                                lhsT=TMP[:, gg * C:gg * C + 32],
                                rhs=W["wB" + nm][:, :], start=True, stop=True,
                                tile_position=(0, e * 32))
                        Wi = gb * C + t  # global 128-target window
                        nc.vector.tensor_add(out=u_sb[:, Wi * Cp:(Wi + 1) * Cp],
                                             in0=pu[:, :], in1=W["bb" + nm][:, :Cp])

                # --- beta: gather + max + assemble h' ---
                for j in range(nch):
                    gout = goutp.tile([128, 2560], f32, tag="g")
                    nc.gpsimd.ap_gather(
                        out_ap=gout[:, :], in_ap=Gsrc[:, :],
                        idxs_ap=idxt[:, j * 160:(j + 1) * 160],
                        channels=128, num_elems=N, d=1, num_idxs=2560)
                    m = work.tile([128, 128], f32, tag="m")
                    nc.vector.tensor_reduce(
                        out=m[:, :],
                        in_=gout[:, :].rearrange("p (n k) -> p n k", k=20),
                        axis=mybir.AxisListType.X, op=mybir.AluOpType.max)
                    for a in range(4):
                        pmt = psum.tile([128, 32], f32, tag="trm")
                        nc.tensor.transpose(out=pmt[:, :], in_=m[a * 32:(a + 1) * 32, :],
                                            identity=ident[a * 32:(a + 1) * 32,
                                                           a * 32:(a + 1) * 32],
                                            tile_position=(a * 32, 0))
                        for w2 in range(32 // Cp):
                            g = a * (32 // Cp) + w2
                            Wi = g * nch + j
                            hw = hT[:, Wi * Cp:(Wi + 1) * Cp]
                            nc.vector.tensor_add(out=hw,
                                                 in0=pmt[:, w2 * Cp:(w2 + 1) * Cp],
                                                 in1=u_sb[:, Wi * Cp:(Wi + 1) * Cp])
                            lrelu(hw, hw)
                            if li < 4:
                                nc.sync.dma_start(
                                    out=dst_hTd[Wi * 128:(Wi + 1) * 128, :Cp], in_=hw)

            # ---------- fin ----------
            for i in range(NB):
                ptr = psum.tile([32, 128], f32, tag="tr")
                nc.tensor.transpose(out=ptr[:, :], in_=hT[:, i * 32:(i + 1) * 32],
                                    identity=ident[:, :])
                hc = work.tile([32, 128], f32, tag="hc")
                nc.vector.tensor_copy(out=hc[:, :], in_=ptr[:, :])
                pm = psum.tile([128, 32], f32, tag="mm")
                nc.tensor.matmul(out=pm[:, :], lhsT=hc[:, :], rhs=W["wAfin"][:, :],
                                 start=True, stop=True)
                ot = work.tile([128, 32], f32, tag="t1")
                nc.vector.tensor_add(out=ot[:, :], in0=pm[:, :], in1=W["bbfin"][:, :])
                lrelu(ot[:, :], ot[:, :])
                nc.sync.dma_start(out=out_d[i * 128:(i + 1) * 128, :], in_=ot[:, :])

    nc.compile()
    return nc


def _fold(p):
    """Fold conv bias + eval-BN into (W', b'): y = W'x + b'."""
    W = np.asarray(p["W"], np.float32)
    b = np.asarray(p["b"], np.float32)
    s = (np.asarray(p["gamma"], np.float32)
         / np.sqrt(np.asarray(p["var"], np.float32) + EPS))
    beta = np.asarray(p["beta"], np.float32)
    mean = np.asarray(p["mean"], np.float32)
    return (W * s[:, None]), (b - mean) * s + beta


def _wrap_stream(stream):
    """[2560] -> [16, 160] int16 wrapped (idx i -> partition i%16, col i//16)."""
    return stream.reshape(160, 16).T.astype(np.int16)


def kernel(x, knn, params):
    x = np.asarray(x, np.float32)
    knn_np = np.asarray(knn).astype(np.int64)

    nc = _build_nc()

    # host-side folded weights (shared by all cores)
    common = {}
    for nm, key in [("p1", "pe1"), ("p2", "pe2"), ("fin", "fin")]:
        Wf, bf = _fold(params[key])
        common["wA" + nm] = np.ascontiguousarray(Wf.T)          # [C, C']
        common["bb" + nm] = np.tile(bf[None, :], (128, 1)).astype(np.float32)
    for i, key in enumerate(["ec1", "ec2", "ec3", "ec4", "ec5"]):
        nm = f"e{i + 1}"
        p = params[key]
        Wf, bf = _fold(p)
        C2 = Wf.shape[1] // 2
        A = Wf[:, :C2]                 # applies to gathered features
        Bm = Wf[:, C2:] - Wf[:, :C2]   # applies to xc
        common["wA" + nm] = np.ascontiguousarray(A.T)
        common["wB" + nm] = np.ascontiguousarray(Bm.T)
        common["bb" + nm] = np.tile(bf[None, :], (128, 1)).astype(np.float32)
    common["ident"] = np.eye(128, dtype=np.float32)

    in_maps = []
    for core in range(8):
        b = core // 2
        m = dict(common)
        # x in [n-part, chunk, c] layout: xT[p, i*3+c] = x[b, c, i*128+p]
        xb = x[b]                                       # [3, N]
        xT = xb.reshape(3, NB, 128).transpose(2, 1, 0)  # [128, NB, 3]
        m["xT"] = np.ascontiguousarray(xT.reshape(128, NB * 3))
        # gather index streams (edge source ids), n-major k-inner
        for name, repn in [("idx8", 8), ("idx4", 4)]:
            blocksz = N // repn
            nch = blocksz // 128
            arr = np.zeros((128, nch * 160), np.int16)
            for g in range(8):
                q = g if repn == 8 else g // 2
                for j in range(nch):
                    n0 = q * blocksz + j * 128
                    stream = knn_np[b, n0:n0 + 128, :].reshape(-1)  # [2560]
                    arr[16 * g:16 * (g + 1), j * 160:(j + 1) * 160] = _wrap_stream(stream)
            m[name] = arr
        in_maps.append(m)

    from concourse.bass_utils import run_bass_kernel_spmd
    res = run_bass_kernel_spmd(nc, in_maps, core_ids=list(range(8)))

    out = np.zeros((B, 32, N), np.float32)
    for b in range(B):
        out[b] = res.results[2 * b]["out"].T
    return out


# revision 13
# speedup vs baseline: 1.2550x; 1.2550x over previous
"""MiniGNN (DGCNN-style edge-conv) Trainium2 Bass kernel.

Math reduction used (BN is eval-mode with mean=0/var=1, LeakyReLU and the
BN affine (positive scale) are monotone, so max_k commutes with them and the
1x1 conv can be applied BEFORE the gather):

  per edge-conv layer (C -> C'):
    t   = A @ h            (A = s*W1, s = gamma/sqrt(1+eps))        [C', N]
    m   = max_k t[:, knn[n, k]]                                     [C', N]
    u   = B @ xc + bias    (B = s*(W2 - W1), bias = s*b)            [C', N]
    h'  = LeakyReLU(m + u)
  where xc[c, n] = h[n % C, c*(N/C) + n//C]  (the torch-faithful buffer
  reinterpret of h^T as [C, N]).

Per-core: batch b = core//2 (pairs duplicate work; each pair member computes
the full batch item).  Gather runs on GPSIMD ap_gather with a d=1
channel-major source replicated 128//C' times across partition blocks, each
block gathering a different quarter/eighth of the target points.
"""
import numpy as np

N = 16384
K = 20
B = 4
EPS = 1e-5
SLOPE = 0.2
NB = N // 128  # 128 chunks of 128 points


def _build_nc():
    import concourse.bass as bass
    import concourse.bacc as bacc
    import concourse.mybir as mybir
    from concourse.tile import TileContext
    from concourse import library_config

    f32 = mybir.dt.float32
    nc = bacc.Bacc("TRN2")

    # ---- DRAM I/O ----
    xT_d = nc.dram_tensor("xT", [128, NB * 3], f32, kind="ExternalInput")
    idx8_d = nc.dram_tensor("idx8", [128, 16 * 160], mybir.dt.int16, kind="ExternalInput")
    idx4_d = nc.dram_tensor("idx4", [128, 32 * 160], mybir.dt.int16, kind="ExternalInput")
    ident_d = nc.dram_tensor("ident", [128, 128], f32, kind="ExternalInput")
    # layer weights (transposed for matmul rhs) + broadcast biases
    LAY = [("p1", 3, 16), ("p2", 16, 16),
           ("e1", 16, 16), ("e2", 16, 16), ("e3", 16, 32), ("e4", 32, 32),
           ("e5", 32, 32), ("fin", 32, 32)]
    wd = {}
    for nm, c, cp in LAY:
        wd["wA" + nm] = nc.dram_tensor("wA" + nm, [c, cp], f32, kind="ExternalInput")
        wd["bb" + nm] = nc.dram_tensor("bb" + nm, [128, cp], f32, kind="ExternalInput")
        if nm.startswith("e"):
            wd["wB" + nm] = nc.dram_tensor("wB" + nm, [c, cp], f32, kind="ExternalInput")
    out_d = nc.dram_tensor("out", [N, 32], f32, kind="ExternalOutput")

    with TileContext(nc) as tc:
        with tc.tile_pool(name="gsrc", bufs=1) as gsrcp, \
             tc.tile_pool(name="big", bufs=1) as bigp, \
             tc.tile_pool(name="wpool", bufs=1) as wpool, \
             tc.tile_pool(name="dram", bufs=1, space="DRAM") as dramp, \
             tc.tile_pool(name="work", bufs=3) as work, \
             tc.tile_pool(name="gout", bufs=2) as goutp, \
             tc.tile_pool(name="psum", bufs=1, space="PSUM") as psum:

            Gsrc = gsrcp.tile([128, N], f32)           # gather source (replicated t)
            hT = bigp.tile([128, NB * 32], f32)        # h in [n-part, c] layout
            u_sb = bigp.tile([128, NB * 32], f32)      # u term per layer
            idx8 = bigp.tile([128, 16 * 160], mybir.dt.int16)
            idx4 = bigp.tile([128, 32 * 160], mybir.dt.int16)
            xT = bigp.tile([128, NB * 3], f32)
            xT_st = bigp.tile([128, NB * 3], f32)
            ident = wpool.tile([128, 128], f32)
            ident_st = wpool.tile([128, 128], f32)
            nc.sync.dma_start(out=idx8[:, :], in_=idx8_d[:, :])
            nc.sync.dma_start(out=idx4[:, :], in_=idx4_d[:, :])
            nc.sync.dma_start(out=xT_st[:, :], in_=xT_d[:, :])
            nc.sync.dma_start(out=ident_st[:, :], in_=ident_d[:, :])
            # PE-consumed tensors are staged through a DVE copy so every PE
            # instruction carries at most one (DVE) semaphore wait.
            nc.vector.tensor_copy(out=xT[:, :], in_=xT_st[:, :])
            nc.vector.tensor_copy(out=ident[:, :], in_=ident_st[:, :])
            W = {}
            for nm, c, cp in LAY:
                for wn, rows in (("wA", c), ("bb", 128)) + ((("wB", c),) if nm.startswith("e") else ()):
                    key = wn + nm
                    st = wpool.tile([rows, cp], f32, tag=key + "s", name=key + "s")
                    nc.sync.dma_start(out=st[:, :], in_=wd[key][:, :])
                    W[key] = wpool.tile([rows, cp], f32, tag=key, name=key)
                    nc.vector.tensor_copy(out=W[key][:, :], in_=st[:, :])

            hTd = [dramp.tile([N, 32], f32, tag="hTd0", name="hTd0"),
                   dramp.tile([N, 32], f32, tag="hTd1", name="hTd1")]

            def lrelu(out_ap, in_ap):
                nc.vector.scalar_tensor_tensor(
                    out=out_ap, in0=in_ap, scalar=SLOPE, in1=in_ap,
                    op0=mybir.AluOpType.mult, op1=mybir.AluOpType.max)

            # ---------- L0: pe1 + pe2 (pointwise convs) ----------
            for i in range(NB):
                p1 = psum.tile([3, 128], f32, tag="tr", bufs=2)
                nc.tensor.transpose(out=p1[:, :], in_=xT[:, i * 3:(i + 1) * 3],
                                    identity=ident[:, :])
                x3 = work.tile([3, 128], f32, tag="hc")
                nc.vector.tensor_copy(out=x3[:, :], in_=p1[:, :])
                pm = psum.tile([128, 16], f32, tag="mm", bufs=2)
                nc.tensor.matmul(out=pm[:, :], lhsT=x3[:, :], rhs=W["wAp1"][:, :],
                                 start=True, stop=True)
                t1 = work.tile([128, 16], f32, tag="t1")
                nc.vector.tensor_add(out=t1[:, :], in0=pm[:, :], in1=W["bbp1"][:, :16])
                lrelu(t1[:, :], t1[:, :])
                p2 = psum.tile([16, 128], f32, tag="tr", bufs=2)
                nc.tensor.transpose(out=p2[:, :], in_=t1[:, :], identity=ident[:, :])
                h1c = work.tile([16, 128], f32, tag="hc")
                nc.vector.tensor_copy(out=h1c[:, :], in_=p2[:, :])
                pm2 = psum.tile([128, 16], f32, tag="mm", bufs=2)
                nc.tensor.matmul(out=pm2[:, :], lhsT=h1c[:, :], rhs=W["wAp2"][:, :],
                                 start=True, stop=True)
                hw = hT[:, i * 16:(i + 1) * 16]
                nc.vector.tensor_add(out=hw, in0=pm2[:, :], in1=W["bbp2"][:, :16])
                lrelu(hw, hw)
                nc.sync.dma_start(out=hTd[0][i * 128:(i + 1) * 128, :16], in_=hw)

            # ---------- edge-conv layers ----------
            ECS = [("e1", 16, 16), ("e2", 16, 16), ("e3", 16, 32),
                   ("e4", 32, 32), ("e5", 32, 32)]
            for li, (nm, C, Cp) in enumerate(ECS):
                src_hTd = hTd[li % 2]
                dst_hTd = hTd[(li + 1) % 2]
                REP = 128 // Cp
                nch = N // REP // 128       # gather chunks per block
                idxt = idx8 if Cp == 16 else idx4

                # --- alpha: build t = A@h, channel-major, replicated ---
                for i in range(NB):
                    ptr = psum.tile([C, 128], f32, tag="tr", bufs=2)
                    nc.tensor.transpose(out=ptr[:, :], in_=hT[:, i * C:(i + 1) * C],
                                        identity=ident[:, :])
                    hc = work.tile([C, 128], f32, tag="hc")
                    nc.vector.tensor_copy(out=hc[:, :], in_=ptr[:, :])
                    pt = psum.tile([128, Cp], f32, tag="mm", bufs=2)
                    nc.tensor.matmul(out=pt[:, :], lhsT=hc[:, :], rhs=W["wA" + nm][:, :],
                                     start=True, stop=True)
                    tsb = work.tile([128, Cp], f32, tag="t1")
                    nc.vector.tensor_copy(out=tsb[:, :], in_=pt[:, :])
                    ptr2 = psum.tile([Cp, 128], f32, tag="tr2")
                    nc.tensor.transpose(out=ptr2[:, :], in_=tsb[:, :], identity=ident[:, :])
                    nc.vector.tensor_copy(out=Gsrc[0:Cp, i * 128:(i + 1) * 128], in_=ptr2[:, :])
                for rep in range(1, REP):
                    nc.sync.dma_start(out=Gsrc[rep * Cp:(rep + 1) * Cp, :], in_=Gsrc[0:Cp, :])

                # --- u term via M_g trick: u[gC+r, o] = sum_c h[r, c*(N/C)+g]*B[o,c]
                NG = N // C                  # number of g values
                PER = 128 // C               # g's per psum tile
                hTd_v = src_hTd[:, :C].rearrange("(c g) r -> c g r", c=C)
                for gb in range(NG // 128):
                    TMP = work.tile([C, 128 * C], f32, tag="tmp")
                    nc.sync.dma_start(
                        out=TMP[:, :].rearrange("q (g r) -> q g r", r=C),
                        in_=hTd_v[:, gb * 128:(gb + 1) * 128, :])
                    gpm = 32 // C  # g's per matmul (PE out base must be mult of 32)
                    for t in range(128 // PER):
                        pu = psum.tile([128, Cp], f32, tag="mmu", bufs=2)
                        for e in range(4):
                            gg = t * PER + e * gpm
                            nc.tensor.matmul(
                                out=pu[e * 32:(e + 1) * 32, :],
                                lhsT=TMP[:, gg * C:gg * C + 32],
                                rhs=W["wB" + nm][:, :], start=True, stop=True,
                                tile_position=(0, e * 32))
                        Wi = gb * C + t  # global 128-target window
                        nc.vector.tensor_add(out=u_sb[:, Wi * Cp:(Wi + 1) * Cp],
                                             in0=pu[:, :], in1=W["bb" + nm][:, :Cp])

                # --- beta: gather + max + assemble h' ---
                for j in range(nch):
                    gout = goutp.tile([128, 2560], f32, tag="g")
                    nc.gpsimd.ap_gather(
                        out_ap=gout[:, :], in_ap=Gsrc[:, :],
                        idxs_ap=idxt[:, j * 160:(j + 1) * 160],
                        channels=128, num_elems=N, d=1, num_idxs=2560)
                    m = work.tile([128, 128], f32, tag="m")
                    nc.vector.tensor_reduce(
                        out=m[:, :],
                        in_=gout[:, :].rearrange("p (n k) -> p n k", k=20),
                        axis=mybir.AxisListType.X, op=mybir.AluOpType.max)
                    for a in range(4):
                        pmt = psum.tile([128, 32], f32, tag="trm")
                        nc.tensor.transpose(out=pmt[:, :], in_=m[a * 32:(a + 1) * 32, :],
                                            identity=ident[a * 32:(a + 1) * 32,
                                                           a * 32:(a + 1) * 32],
                                            tile_position=(a * 32, 0))
                        for w2 in range(32 // Cp):
                            g = a * (32 // Cp) + w2
                            Wi = g * nch + j
                            hw = hT[:, Wi * Cp:(Wi + 1) * Cp]
                            nc.vector.tensor_add(out=hw,
                                                 in0=pmt[:, w2 * Cp:(w2 + 1) * Cp],
                                                 in1=u_sb[:, Wi * Cp:(Wi + 1) * Cp])
                            lrelu(hw, hw)
                            if li < 4:
                                nc.sync.dma_start(
                                    out=dst_hTd[Wi * 128:(Wi + 1) * 128, :Cp], in_=hw)

            # ---------- fin ----------
            for i in range(NB):
                ptr = psum.tile([32, 128], f32, tag="tr", bufs=2)
                nc.tensor.transpose(out=ptr[:, :], in_=hT[:, i * 32:(i + 1) * 32],
                                    identity=ident[:, :])
                hc = work.tile([32, 128], f32, tag="hc")
                nc.vector.tensor_copy(out=hc[:, :], in_=ptr[:, :])
                pm = psum.tile([128, 32], f32, tag="mm", bufs=2)
                nc.tensor.matmul(out=pm[:, :], lhsT=hc[:, :], rhs=W["wAfin"][:, :],
                                 start=True, stop=True)
                ot = work.tile([128, 32], f32, tag="t1")
                nc.vector.tensor_add(out=ot[:, :], in0=pm[:, :], in1=W["bbfin"][:, :])
                lrelu(ot[:, :], ot[:, :])
                nc.sync.dma_start(out=out_d[i * 128:(i + 1) * 128, :], in_=ot[:, :])

    nc.compile()
    return nc


def _fold(p):
    """Fold conv bias + eval-BN into (W', b'): y = W'x + b'."""
    W = np.asarray(p["W"], np.float32)
    b = np.asarray(p["b"], np.float32)
    s = (np.asarray(p["gamma"], np.float32)
         / np.sqrt(np.asarray(p["var"], np.float32) + EPS))
    beta = np.asarray(p["beta"], np.float32)
    mean = np.asarray(p["mean"], np.float32)
    return (W * s[:, None]), (b - mean) * s + beta


def _wrap_stream(stream):
    """[2560] -> [16, 160] int16 wrapped (idx i -> partition i%16, col i//16)."""
    return stream.reshape(160, 16).T.astype(np.int16)


def kernel(x, knn, params):
    x = np.asarray(x, np.float32)
    knn_np = np.asarray(knn).astype(np.int64)

    nc = _build_nc()

    # host-side folded weights (shared by all cores)
    common = {}
    for nm, key in [("p1", "pe1"), ("p2", "pe2"), ("fin", "fin")]:
        Wf, bf = _fold(params[key])
        common["wA" + nm] = np.ascontiguousarray(Wf.T)          # [C, C']
        common["bb" + nm] = np.tile(bf[None, :], (128, 1)).astype(np.float32)
    for i, key in enumerate(["ec1", "ec2", "ec3", "ec4", "ec5"]):
        nm = f"e{i + 1}"
        p = params[key]
        Wf, bf = _fold(p)
        C2 = Wf.shape[1] // 2
        A = Wf[:, :C2]                 # applies to gathered features
        Bm = Wf[:, C2:] - Wf[:, :C2]   # applies to xc
        common["wA" + nm] = np.ascontiguousarray(A.T)
        common["wB" + nm] = np.ascontiguousarray(Bm.T)
        common["bb" + nm] = np.tile(bf[None, :], (128, 1)).astype(np.float32)
    common["ident"] = np.eye(128, dtype=np.float32)

    in_maps = []
    for core in range(8):
        b = core // 2
        m = dict(common)
        # x in [n-part, chunk, c] layout: xT[p, i*3+c] = x[b, c, i*128+p]
        xb = x[b]                                       # [3, N]
        xT = xb.reshape(3, NB, 128).transpose(2, 1, 0)  # [128, NB, 3]
        m["xT"] = np.ascontiguousarray(xT.reshape(128, NB * 3))
        # gather index streams (edge source ids), n-major k-inner
        for name, repn in [("idx8", 8), ("idx4", 4)]:
            blocksz = N // repn
            nch = blocksz // 128
            arr = np.zeros((128, nch * 160), np.int16)
            for g in range(8):
                q = g if repn == 8 else g // 2
                for j in range(nch):
                    n0 = q * blocksz + j * 128
                    stream = knn_np[b, n0:n0 + 128, :].reshape(-1)  # [2560]
                    arr[16 * g:16 * (g + 1), j * 160:(j + 1) * 160] = _wrap_stream(stream)
            m[name] = arr
        in_maps.append(m)

    from concourse.bass_utils import run_bass_kernel_spmd
    res = run_bass_kernel_spmd(nc, in_maps, core_ids=list(range(8)))

    out = np.zeros((B, 32, N), np.float32)
    for b in range(B):
        out[b] = res.results[2 * b]["out"].T
    return out


# revision 14
# speedup vs baseline: 2.5896x; 2.0633x over previous
"""MiniGNN (DGCNN-style edge-conv) Trainium2 Bass kernel.

Math reduction used (BN is eval-mode with mean=0/var=1, LeakyReLU and the
BN affine (positive scale) are monotone, so max_k commutes with them and the
1x1 conv can be applied BEFORE the gather):

  per edge-conv layer (C -> C'):
    t   = A @ h            (A = s*W1, s = gamma/sqrt(1+eps))        [C', N]
    m   = max_k t[:, knn[n, k]]                                     [C', N]
    u   = B @ xc + bias    (B = s*(W2 - W1), bias = s*b)            [C', N]
    h'  = LeakyReLU(m + u)
  where xc[c, n] = h[n % C, c*(N/C) + n//C]  (the torch-faithful buffer
  reinterpret of h^T as [C, N]).

Per-core: batch b = core//2 (pairs duplicate work; each pair member computes
the full batch item).  Gather runs on GPSIMD ap_gather with a d=1
channel-major source replicated 128//C' times across partition blocks, each
block gathering a different quarter/eighth of the target points.
"""
import numpy as np

N = 16384
K = 20
B = 4
EPS = 1e-5
SLOPE = 0.2
NB = N // 128  # 128 chunks of 128 points

_NC_CACHE = None


def _build_nc():
    import concourse.bass as bass
    import concourse.bacc as bacc
    import concourse.mybir as mybir
    from concourse.tile import TileContext
    from concourse import library_config

    f32 = mybir.dt.float32
    nc = bacc.Bacc("TRN2")

    # ---- DRAM I/O ----
    xT_d = nc.dram_tensor("xT", [128, NB * 3], f32, kind="ExternalInput")
    idx8_d = nc.dram_tensor("idx8", [128, 16 * 160], mybir.dt.int16, kind="ExternalInput")
    idx4_d = nc.dram_tensor("idx4", [128, 32 * 160], mybir.dt.int16, kind="ExternalInput")
    ident_d = nc.dram_tensor("ident", [128, 128], f32, kind="ExternalInput")
    # layer weights (transposed for matmul rhs) + broadcast biases
    LAY = [("p1", 3, 16), ("p2", 16, 16),
           ("e1", 16, 16), ("e2", 16, 16), ("e3", 16, 32), ("e4", 32, 32),
           ("e5", 32, 32), ("fin", 32, 32)]
    wd = {}
    for nm, c, cp in LAY:
        wd["wA" + nm] = nc.dram_tensor("wA" + nm, [c, cp], f32, kind="ExternalInput")
        wd["bb" + nm] = nc.dram_tensor("bb" + nm, [128, cp], f32, kind="ExternalInput")
        if nm.startswith("e"):
            wd["wB" + nm] = nc.dram_tensor("wB" + nm, [c, cp], f32, kind="ExternalInput")
    out_d = nc.dram_tensor("out", [N, 32], f32, kind="ExternalOutput")

    with TileContext(nc) as tc:
        with tc.tile_pool(name="gsrc", bufs=1) as gsrcp, \
             tc.tile_pool(name="big", bufs=1) as bigp, \
             tc.tile_pool(name="wpool", bufs=1) as wpool, \
             tc.tile_pool(name="dram", bufs=1, space="DRAM") as dramp, \
             tc.tile_pool(name="work", bufs=3) as work, \
             tc.tile_pool(name="gout", bufs=2) as goutp, \
             tc.tile_pool(name="psum", bufs=1, space="PSUM") as psum:

            Gsrc = gsrcp.tile([128, N], f32)           # gather source (replicated t)
            hT = bigp.tile([128, NB * 32], f32)        # h in [n-part, c] layout
            u_sb = bigp.tile([128, NB * 32], f32)      # u term per layer
            idx8 = bigp.tile([128, 16 * 160], mybir.dt.int16)
            idx4 = bigp.tile([128, 32 * 160], mybir.dt.int16)
            xT = bigp.tile([128, NB * 3], f32)
            xT_st = bigp.tile([128, NB * 3], f32)
            ident = wpool.tile([128, 128], f32)
            ident_st = wpool.tile([128, 128], f32)
            nc.sync.dma_start(out=idx8[:, :], in_=idx8_d[:, :])
            nc.sync.dma_start(out=idx4[:, :], in_=idx4_d[:, :])
            nc.sync.dma_start(out=xT_st[:, :], in_=xT_d[:, :])
            nc.sync.dma_start(out=ident_st[:, :], in_=ident_d[:, :])
            # PE-consumed tensors are staged through a DVE copy so every PE
            # instruction carries at most one (DVE) semaphore wait.
            nc.vector.tensor_copy(out=xT[:, :], in_=xT_st[:, :])
            nc.vector.tensor_copy(out=ident[:, :], in_=ident_st[:, :])
            W = {}
            for nm, c, cp in LAY:
                for wn, rows in (("wA", c), ("bb", 128)) + ((("wB", c),) if nm.startswith("e") else ()):
                    key = wn + nm
                    st = wpool.tile([rows, cp], f32, tag=key + "s", name=key + "s")
                    nc.sync.dma_start(out=st[:, :], in_=wd[key][:, :])
                    W[key] = wpool.tile([rows, cp], f32, tag=key, name=key)
                    nc.vector.tensor_copy(out=W[key][:, :], in_=st[:, :])

            hTd = [dramp.tile([N, 32], f32, tag="hTd0", name="hTd0"),
                   dramp.tile([N, 32], f32, tag="hTd1", name="hTd1")]

            def lrelu(out_ap, in_ap):
                nc.vector.scalar_tensor_tensor(
                    out=out_ap, in0=in_ap, scalar=SLOPE, in1=in_ap,
                    op0=mybir.AluOpType.mult, op1=mybir.AluOpType.max)

            # ---------- L0: pe1 + pe2 (pointwise convs) ----------
            for i in range(NB):
                p1 = psum.tile([3, 128], f32, tag="tr", bufs=2)
                nc.tensor.transpose(out=p1[:, :], in_=xT[:, i * 3:(i + 1) * 3],
                                    identity=ident[:, :])
                x3 = work.tile([3, 128], f32, tag="hc")
                nc.vector.tensor_copy(out=x3[:, :], in_=p1[:, :])
                pm = psum.tile([128, 16], f32, tag="mm", bufs=2)
                nc.tensor.matmul(out=pm[:, :], lhsT=x3[:, :], rhs=W["wAp1"][:, :],
                                 start=True, stop=True)
                t1 = work.tile([128, 16], f32, tag="t1")
                nc.vector.tensor_add(out=t1[:, :], in0=pm[:, :], in1=W["bbp1"][:, :16])
                lrelu(t1[:, :], t1[:, :])
                p2 = psum.tile([16, 128], f32, tag="tr", bufs=2)
                nc.tensor.transpose(out=p2[:, :], in_=t1[:, :], identity=ident[:, :])
                h1c = work.tile([16, 128], f32, tag="hc")
                nc.vector.tensor_copy(out=h1c[:, :], in_=p2[:, :])
                pm2 = psum.tile([128, 16], f32, tag="mm", bufs=2)
                nc.tensor.matmul(out=pm2[:, :], lhsT=h1c[:, :], rhs=W["wAp2"][:, :],
                                 start=True, stop=True)
                hw = hT[:, i * 16:(i + 1) * 16]
                nc.vector.tensor_add(out=hw, in0=pm2[:, :], in1=W["bbp2"][:, :16])
                lrelu(hw, hw)
                nc.sync.dma_start(out=hTd[0][i * 128:(i + 1) * 128, :16], in_=hw)

            # ---------- edge-conv layers ----------
            ECS = [("e1", 16, 16), ("e2", 16, 16), ("e3", 16, 32),
                   ("e4", 32, 32), ("e5", 32, 32)]
            for li, (nm, C, Cp) in enumerate(ECS):
                src_hTd = hTd[li % 2]
                dst_hTd = hTd[(li + 1) % 2]
                REP = 128 // Cp
                nch = N // REP // 128       # gather chunks per block
                idxt = idx8 if Cp == 16 else idx4

                # --- alpha: build t = A@h, channel-major, replicated ---
                for i in range(NB):
                    ptr = psum.tile([C, 128], f32, tag="tr", bufs=2)
                    nc.tensor.transpose(out=ptr[:, :], in_=hT[:, i * C:(i + 1) * C],
                                        identity=ident[:, :])
                    hc = work.tile([C, 128], f32, tag="hc")
                    nc.vector.tensor_copy(out=hc[:, :], in_=ptr[:, :])
                    pt = psum.tile([128, Cp], f32, tag="mm", bufs=2)
                    nc.tensor.matmul(out=pt[:, :], lhsT=hc[:, :], rhs=W["wA" + nm][:, :],
                                     start=True, stop=True)
                    tsb = work.tile([128, Cp], f32, tag="t1")
                    nc.vector.tensor_copy(out=tsb[:, :], in_=pt[:, :])
                    ptr2 = psum.tile([Cp, 128], f32, tag="tr2")
                    nc.tensor.transpose(out=ptr2[:, :], in_=tsb[:, :], identity=ident[:, :])
                    nc.vector.tensor_copy(out=Gsrc[0:Cp, i * 128:(i + 1) * 128], in_=ptr2[:, :])
                for rep in range(1, REP):
                    nc.sync.dma_start(out=Gsrc[rep * Cp:(rep + 1) * Cp, :], in_=Gsrc[0:Cp, :])

                # --- u term via M_g trick: u[gC+r, o] = sum_c h[r, c*(N/C)+g]*B[o,c]
                NG = N // C                  # number of g values
                PER = 128 // C               # g's per psum tile
                hTd_v = src_hTd[:, :C].rearrange("(c g) r -> c g r", c=C)
                for gb in range(NG // 128):
                    TMP = work.tile([C, 128 * C], f32, tag="tmp")
                    nc.sync.dma_start(
                        out=TMP[:, :].rearrange("q (g r) -> q g r", r=C),
                        in_=hTd_v[:, gb * 128:(gb + 1) * 128, :])
                    gpm = 32 // C  # g's per matmul (PE out base must be mult of 32)
                    for t in range(128 // PER):
                        pu = psum.tile([128, Cp], f32, tag="mmu", bufs=2)
                        for e in range(4):
                            gg = t * PER + e * gpm
                            nc.tensor.matmul(
                                out=pu[e * 32:(e + 1) * 32, :],
                                lhsT=TMP[:, gg * C:gg * C + 32],
                                rhs=W["wB" + nm][:, :], start=True, stop=True,
                                tile_position=(0, e * 32))
                        Wi = gb * C + t  # global 128-target window
                        nc.vector.tensor_add(out=u_sb[:, Wi * Cp:(Wi + 1) * Cp],
                                             in0=pu[:, :], in1=W["bb" + nm][:, :Cp])

                # --- beta: gather + max + assemble h' ---
                for j in range(nch):
                    gout = goutp.tile([128, 2560], f32, tag="g")
                    nc.gpsimd.ap_gather(
                        out_ap=gout[:, :], in_ap=Gsrc[:, :],
                        idxs_ap=idxt[:, j * 160:(j + 1) * 160],
                        channels=128, num_elems=N, d=1, num_idxs=2560)
                    m = work.tile([128, 128], f32, tag="m")
                    nc.vector.tensor_reduce(
                        out=m[:, :],
                        in_=gout[:, :].rearrange("p (n k) -> p n k", k=20),
                        axis=mybir.AxisListType.X, op=mybir.AluOpType.max)
                    for a in range(4):
                        pmt = psum.tile([128, 32], f32, tag="trm")
                        nc.tensor.transpose(out=pmt[:, :], in_=m[a * 32:(a + 1) * 32, :],
                                            identity=ident[a * 32:(a + 1) * 32,
                                                           a * 32:(a + 1) * 32],
                                            tile_position=(a * 32, 0))
                        for w2 in range(32 // Cp):
                            g = a * (32 // Cp) + w2
                            Wi = g * nch + j
                            hw = hT[:, Wi * Cp:(Wi + 1) * Cp]
                            nc.vector.tensor_add(out=hw,
                                                 in0=pmt[:, w2 * Cp:(w2 + 1) * Cp],
                                                 in1=u_sb[:, Wi * Cp:(Wi + 1) * Cp])
                            lrelu(hw, hw)
                            if li < 4:
                                nc.sync.dma_start(
                                    out=dst_hTd[Wi * 128:(Wi + 1) * 128, :Cp], in_=hw)

            # ---------- fin ----------
            for i in range(NB):
                ptr = psum.tile([32, 128], f32, tag="tr", bufs=2)
                nc.tensor.transpose(out=ptr[:, :], in_=hT[:, i * 32:(i + 1) * 32],
                                    identity=ident[:, :])
                hc = work.tile([32, 128], f32, tag="hc")
                nc.vector.tensor_copy(out=hc[:, :], in_=ptr[:, :])
                pm = psum.tile([128, 32], f32, tag="mm", bufs=2)
                nc.tensor.matmul(out=pm[:, :], lhsT=hc[:, :], rhs=W["wAfin"][:, :],
                                 start=True, stop=True)
                ot = work.tile([128, 32], f32, tag="t1")
                nc.vector.tensor_add(out=ot[:, :], in0=pm[:, :], in1=W["bbfin"][:, :])
                lrelu(ot[:, :], ot[:, :])
                nc.sync.dma_start(out=out_d[i * 128:(i + 1) * 128, :], in_=ot[:, :])

    nc.compile()
    return nc


def _fold(p):
    """Fold conv bias + eval-BN into (W', b'): y = W'x + b'."""
    W = np.asarray(p["W"], np.float32)
    b = np.asarray(p["b"], np.float32)
    s = (np.asarray(p["gamma"], np.float32)
         / np.sqrt(np.asarray(p["var"], np.float32) + EPS))
    beta = np.asarray(p["beta"], np.float32)
    mean = np.asarray(p["mean"], np.float32)
    return (W * s[:, None]), (b - mean) * s + beta


def _wrap_stream(stream):
    """[2560] -> [16, 160] int16 wrapped (idx i -> partition i%16, col i//16)."""
    return stream.reshape(160, 16).T.astype(np.int16)


def kernel(x, knn, params):
    global _NC_CACHE
    x = np.asarray(x, np.float32)
    knn_np = np.asarray(knn).astype(np.int64)

    if _NC_CACHE is None:
        _NC_CACHE = _build_nc()
    nc = _NC_CACHE

    # host-side folded weights (shared by all cores)
    common = {}
    for nm, key in [("p1", "pe1"), ("p2", "pe2"), ("fin", "fin")]:
        Wf, bf = _fold(params[key])
        common["wA" + nm] = np.ascontiguousarray(Wf.T)          # [C, C']
        common["bb" + nm] = np.tile(bf[None, :], (128, 1)).astype(np.float32)
    for i, key in enumerate(["ec1", "ec2", "ec3", "ec4", "ec5"]):
        nm = f"e{i + 1}"
        p = params[key]
        Wf, bf = _fold(p)
        C2 = Wf.shape[1] // 2
        A = Wf[:, :C2]                 # applies to gathered features
        Bm = Wf[:, C2:] - Wf[:, :C2]   # applies to xc
        common["wA" + nm] = np.ascontiguousarray(A.T)
        common["wB" + nm] = np.ascontiguousarray(Bm.T)
        common["bb" + nm] = np.tile(bf[None, :], (128, 1)).astype(np.float32)
    common["ident"] = np.eye(128, dtype=np.float32)

    in_maps = []
    for core in range(8):
        b = core // 2
        m = dict(common)
        # x in [n-part, chunk, c] layout: xT[p, i*3+c] = x[b, c, i*128+p]
        xb = x[b]                                       # [3, N]
        xT = xb.reshape(3, NB, 128).transpose(2, 1, 0)  # [128, NB, 3]
        m["xT"] = np.ascontiguousarray(xT.reshape(128, NB * 3))
        # gather index streams (edge source ids), n-major k-inner
        for name, repn in [("idx8", 8), ("idx4", 4)]:
            blocksz = N // repn
            nch = blocksz // 128
            arr = np.zeros((128, nch * 160), np.int16)
            for g in range(8):
                q = g if repn == 8 else g // 2
                for j in range(nch):
                    n0 = q * blocksz + j * 128
                    stream = knn_np[b, n0:n0 + 128, :].reshape(-1)  # [2560]
                    arr[16 * g:16 * (g + 1), j * 160:(j + 1) * 160] = _wrap_stream(stream)
            m[name] = arr
        in_maps.append(m)

    from concourse.bass_utils import run_bass_kernel_spmd
    res = run_bass_kernel_spmd(nc, in_maps, core_ids=list(range(8)))

    out = np.zeros((B, 32, N), np.float32)
    for b in range(B):
        out[b] = res.results[2 * b]["out"].T
    return out
